# revision 13
# baseline (speedup 1.0000x reference)
"""Trainium2 Bass kernel for CoocOpModel.

out[b,s,z] = sum_{i,j} func[b,s,i] * cooc[i,j,z] * arg[b,s,j]
  with func = func_and_arg[..., :128], arg = func_and_arg[..., 128:]

Shapes (hardcoded): func_and_arg [4,1024,256] f32, cooccurrences [128,128,128] f32,
out [4,1024,128] f32.  D = 128, tokens T = 4096.

Strategy: data-parallel over tokens across 8 cores (512 tokens/core).

Per-core math as ONE flattened contraction over (i,j):
  out[z, t] = sum_{(i,j)} C2[(i,j), z] * P[(i,j), t],  P[(i,j), t] = f[i,t]*a[j,t]

The 16384-long (i,j) axis is processed as 128 PSUM-accumulated matmul
chunks of 128 partition-pairs each.  A chunk covers GI=8 i's x GJ=16 j's
(partition p = ii*16 + jj).  The mixed layout keeps the moving-operand
build cheap: per chunk, P = f_slab * a_slab is a plain DVE
tensor_tensor multiply of two replicated slabs.

The slabs are PRE-REPLICATED ON THE HOST and DMA'd as dense contiguous
copies (2-4KB descriptor rows), which roughly doubles effective queue
bandwidth vs. step-0 broadcast APs:
  fr[p, I*512+t] = f[I*8 + p//16, t]   (2MB)
  ar[p, J*512+t] = a[J*16 + p%16, t]   (1MB)

The first chunk runs off a small dedicated a_j0 tile so the PE chain
starts after ~380KB of DMA instead of ~900KB.

PE: 128 matmuls, stationary = c2r chunk [p=128, z=128], moving = P
[p=128, t=512], all accumulating into one PSUM bank [128z, 512t] f32.

Host pre-reorder: c2r[ii*16+jj, (I*8+J)*128 + z] = cooc[I*8+ii, J*16+jj, z].
"""

import sys

sys.path.insert(0, "/opt/trn_rl_repo")

import numpy as np
from contextlib import ExitStack

import concourse.bass as bass
import concourse.tile as tile
from concourse import bacc, mybir
from concourse.bass_utils import run_bass_kernel_spmd

F16 = mybir.dt.float16
F32 = mybir.dt.float32
NP_F16 = np.float16

N_CORES = 8
D = 128
T_TOTAL = 4096
T_CORE = T_TOTAL // N_CORES  # 512
GI, GJ = 8, 16               # i's / j's per chunk
NI, NJ = D // GI, D // GJ    # 16 I-groups, 8 J-groups
HALF = 4 * T_CORE            # 2048

_NC_CACHE = None


def _build():
    nc = bacc.Bacc("TRN2", target_bir_lowering=False, debug=False, num_devices=N_CORES)

    fr_d = nc.dram_tensor("fr", [D, NI * T_CORE], F16, kind="ExternalInput").ap()
    ar_d = nc.dram_tensor("ar", [D, NJ * T_CORE], F16, kind="ExternalInput").ap()
    # c2r[ii*16+jj, (I*8+J)*128 + z] = cooc[I*8+ii, J*16+jj, z]
    c2 = nc.dram_tensor("c2", [D, D * D], F16, kind="ExternalInput").ap()
    out_t = nc.dram_tensor("out_t", [D, T_CORE], F32, kind="ExternalOutput").ap()

    with tile.TileContext(nc) as tc:
        with ExitStack() as ctx:
            const_pool = ctx.enter_context(tc.tile_pool(name="const", bufs=1))
            c_pool = ctx.enter_context(tc.tile_pool(name="csl", bufs=4))
            p_pool = ctx.enter_context(tc.tile_pool(name="p", bufs=4))
            psum_pool = ctx.enter_context(
                tc.tile_pool(name="psum", bufs=1, space="PSUM")
            )

            # --- head DMAs ---
            # sync queue: a tiles (dense copies of the prereplicated image);
            # a_half1 goes on the scalar queue so both a halves land in
            # parallel; a_full (used from I=2 on) streams in behind them
            a_j0 = const_pool.tile([D, T_CORE], F16, tag="aj0")
            nc.sync.dma_start(a_j0[:], ar_d[:, 0:T_CORE])
            a_half0 = const_pool.tile([D, HALF], F16, tag="a0")
            nc.sync.dma_start(a_half0[:], ar_d[:, 0:HALF])
            a_half1 = const_pool.tile([D, HALF], F16, tag="a1")
            nc.scalar.dma_start(a_half1[:], ar_d[:, HALF : 2 * HALF])
            a_halves = [a_half0, a_half1]
            a_full = const_pool.tile([D, 2 * HALF], F16, tag="af")
            nc.sync.dma_start(a_full[:], ar_d[:, 0 : 2 * HALF])

            # scalar queue: f slab tiles (2 I-groups each) and cooc slabs
            fr_tiles = {}

            def fr_dma(k):
                frk = const_pool.tile([D, 2 * T_CORE], F16, tag=f"fr{k}")
                nc.scalar.dma_start(
                    frk[:], fr_d[:, k * 2 * T_CORE : (k + 1) * 2 * T_CORE]
                )
                fr_tiles[k] = frk

            c_slabs = {}

            def c_dma(I, eng):
                c_sb = c_pool.tile([D, NJ * D], F16, tag=f"c{I}")
                eng.dma_start(c_sb[:], c2[:, I * NJ * D : (I + 1) * NJ * D])
                c_slabs[I] = c_sb

            fr_dma(0)
            c_dma(0, nc.scalar)
            fr_dma(1)

            ps = psum_pool.tile([D, T_CORE], F32)

            q = 0
            for I in range(NI):
                k = I // 2
                if k not in fr_tiles:
                    fr_dma(k)
                if I % 2 == 0 and (k + 1) not in fr_tiles and k + 1 < NI // 2:
                    fr_dma(k + 1)
                if I + 1 < NI:
                    c_dma(I + 1, nc.sync if I % 2 == 0 else nc.scalar)
                c_sb = c_slabs.pop(I)
                fr_sl = fr_tiles[k][:, (I % 2) * T_CORE : (I % 2 + 1) * T_CORE]

                def f_view(reps):
                    return bass.AP(
                        fr_sl.tensor,
                        fr_sl.offset,
                        [fr_sl.ap[0], [0, reps], [1, T_CORE]],
                    )

                def emit_unit(Jbase, cnt, a_ap, tag=None):
                    nonlocal q
                    if tag is None:
                        pt = p_pool.tile([D, cnt * T_CORE], F16, tag="p")
                    else:
                        pt = const_pool.tile([D, cnt * T_CORE], F16, tag=tag)
                    nc.vector.tensor_mul(pt[:], f_view(cnt), a_ap)
                    for kk in range(cnt):
                        J = Jbase + kk
                        nc.tensor.matmul(
                            ps[:],
                            c_sb[:, J * D : (J + 1) * D],
                            pt[:, kk * T_CORE : (kk + 1) * T_CORE],
                            start=(q == 0),
                            stop=(q == NI * NJ - 1),
                        )
                        q += 1

                if I == 0:
                    # tiny first unit off the small a_j0 tile, then the rest
                    # of the group off the two a halves
                    emit_unit(0, 1, a_j0[:], tag="p0")
                    emit_unit(1, 3, a_half0[:, T_CORE : 4 * T_CORE], tag="p1")
                    emit_unit(4, 4, a_half1[:])
                elif I == 1:
                    emit_unit(0, 4, a_half0[:])
                    emit_unit(4, 4, a_half1[:])
                elif I == NI - 1:
                    # taper the last group so the PE chain ends right after
                    # the final (single-chunk) TT
                    emit_unit(0, 7, a_full[:, 0 : 7 * T_CORE])
                    emit_unit(7, 1, a_full[:, 7 * T_CORE : 8 * T_CORE], tag="pz")
                else:
                    emit_unit(0, 8, a_full[:])

            o_sb = const_pool.tile([D, T_CORE], F32, tag="o")
            nc.vector.tensor_copy(o_sb[:], ps[:])
            nc.sync.dma_start(out_t[:, :], o_sb[:])

    nc.compile()
    return nc


def _get_nc():
    global _NC_CACHE
    if _NC_CACHE is None:
        _NC_CACHE = _build()
    return _NC_CACHE


def _prep_in_maps(func_and_arg, cooccurrences):
    fa = np.asarray(func_and_arg, dtype=np.float32).reshape(T_TOTAL, 2 * D)
    c2r = (
        np.asarray(cooccurrences, dtype=np.float32)
        .reshape(NI, GI, NJ, GJ, D)
        .transpose(1, 3, 0, 2, 4)
        .reshape(D, D * D)
        .astype(NP_F16)
    )
    c2r = np.ascontiguousarray(c2r)
    in_maps = []
    for c in range(N_CORES):
        s = fa[c * T_CORE : (c + 1) * T_CORE]  # [512, 256]
        f_tc = np.ascontiguousarray(s[:, :D].T).astype(NP_F16)  # [128 i, 512 t]
        a_tc = np.ascontiguousarray(s[:, D:].T).astype(NP_F16)  # [128 j, 512 t]
        # fr[p, I*512+t] = f[I*8 + p//16, t]
        fr = np.ascontiguousarray(
            np.broadcast_to(
                f_tc.reshape(NI, GI, 1, T_CORE), (NI, GI, GJ, T_CORE)
            ).transpose(1, 2, 0, 3).reshape(D, NI * T_CORE)
        )
        # ar[p, J*512+t] = a[J*16 + p%16, t]
        ar = np.ascontiguousarray(
            np.broadcast_to(
                a_tc.reshape(1, NJ, GJ, T_CORE).transpose(0, 2, 1, 3),
                (GI, GJ, NJ, T_CORE),
            ).reshape(D, NJ * T_CORE)
        )
        in_maps.append({"fr": fr, "ar": ar, "c2": c2r})
    return in_maps


def kernel(func_and_arg: np.ndarray, cooccurrences: np.ndarray) -> np.ndarray:
    assert func_and_arg.shape == (4, 1024, 2 * D)
    assert cooccurrences.shape == (D, D, D)

    in_maps = _prep_in_maps(func_and_arg, cooccurrences)
    nc = _get_nc()
    res = run_bass_kernel_spmd(nc, in_maps, core_ids=list(range(N_CORES)))

    # out_t per core: [z=128, t=512] -> [t, z]; concat over cores -> [4096, 128]
    outs = [res.results[c]["out_t"].T for c in range(N_CORES)]
    out = np.concatenate(outs, axis=0).reshape(4, 1024, D).astype(np.float32)
    return out


# revision 15
# speedup vs baseline: 1.1721x; 1.1721x over previous
"""Trainium2 Bass kernel for CoocOpModel.

out[b,s,z] = sum_{i,j} func[b,s,i] * cooc[i,j,z] * arg[b,s,j]
  with func = func_and_arg[..., :128], arg = func_and_arg[..., 128:]

Shapes (hardcoded): func_and_arg [4,1024,256] f32, cooccurrences [128,128,128] f32,
out [4,1024,128] f32.  D = 128, tokens T = 4096.

Strategy: data-parallel over tokens across 8 cores (512 tokens/core).

Per-core math as ONE flattened contraction over (i,j):
  out[z, t] = sum_{(i,j)} C2[(i,j), z] * P[(i,j), t],  P[(i,j), t] = f[i,t]*a[j,t]

The 16384-long (i,j) axis is processed as 128 PSUM-accumulated matmul
chunks of 128 partition-pairs each.  A chunk covers GI=8 i's x GJ=16 j's
(partition p = ii*16 + jj).  The mixed layout keeps the moving-operand
build cheap: per chunk, P = f_slab * a_slab is a plain DVE
tensor_tensor multiply of two replicated slabs.

The slabs are PRE-REPLICATED ON THE HOST and DMA'd as dense contiguous
copies (2-4KB descriptor rows), which roughly doubles effective queue
bandwidth vs. step-0 broadcast APs:
  fr[p, I*512+t] = f[I*8 + p//16, t]   (2MB)
  ar[p, J*512+t] = a[J*16 + p%16, t]   (1MB)

The first chunk runs off a small dedicated a_j0 tile so the PE chain
starts after ~380KB of DMA instead of ~900KB.

PE: 128 matmuls, stationary = c2r chunk [p=128, z=128], moving = P
[p=128, t=512], all accumulating into one PSUM bank [128z, 512t] f32.

Host pre-reorder: c2r[ii*16+jj, (I*8+J)*128 + z] = cooc[I*8+ii, J*16+jj, z].
"""

import sys

sys.path.insert(0, "/opt/trn_rl_repo")

import numpy as np
from contextlib import ExitStack

import concourse.bass as bass
import concourse.tile as tile
from concourse import bacc, mybir
from concourse.bass_utils import run_bass_kernel_spmd

F16 = mybir.dt.float16
F32 = mybir.dt.float32
NP_F16 = np.float16

N_CORES = 8
D = 128
T_TOTAL = 4096
T_CORE = T_TOTAL // N_CORES  # 512
GI, GJ = 8, 16               # i's / j's per chunk
NI, NJ = D // GI, D // GJ    # 16 I-groups, 8 J-groups
HALF = 4 * T_CORE            # 2048

_NC_CACHE = None


def _build():
    nc = bacc.Bacc("TRN2", target_bir_lowering=False, debug=False, num_devices=N_CORES)

    fr_d = nc.dram_tensor("fr", [D, NI * T_CORE], F16, kind="ExternalInput").ap()
    ar_d = nc.dram_tensor("ar", [D, NJ * T_CORE], F16, kind="ExternalInput").ap()
    # c2r[ii*16+jj, (I*8+J)*128 + z] = cooc[I*8+ii, J*16+jj, z]
    c2 = nc.dram_tensor("c2", [D, D * D], F16, kind="ExternalInput").ap()
    out_t = nc.dram_tensor("out_t", [D, T_CORE], F32, kind="ExternalOutput").ap()

    with tile.TileContext(nc) as tc:
        with ExitStack() as ctx:
            const_pool = ctx.enter_context(tc.tile_pool(name="const", bufs=1))
            c_pool = ctx.enter_context(tc.tile_pool(name="csl", bufs=4))
            p_pool = ctx.enter_context(tc.tile_pool(name="p", bufs=4))
            psum_pool = ctx.enter_context(
                tc.tile_pool(name="psum", bufs=1, space="PSUM")
            )

            # --- head DMAs ---
            # sync queue: a tiles (dense copies of the prereplicated image);
            # a_half1 goes on the scalar queue so both a halves land in
            # parallel; a_full (used from I=2 on) streams in behind them
            a_j0 = const_pool.tile([D, T_CORE], F16, tag="aj0")
            nc.sync.dma_start(a_j0[:], ar_d[:, 0:T_CORE])
            a_half0 = const_pool.tile([D, HALF], F16, tag="a0")
            nc.sync.dma_start(a_half0[:], ar_d[:, 0:HALF])
            a_half1 = const_pool.tile([D, HALF], F16, tag="a1")
            nc.scalar.dma_start(a_half1[:], ar_d[:, HALF : 2 * HALF])
            a_halves = [a_half0, a_half1]

            # scalar queue: f slab tiles (2 I-groups each) and cooc slabs
            fr_tiles = {}

            def fr_dma(k):
                frk = const_pool.tile([D, 2 * T_CORE], F16, tag=f"fr{k}")
                nc.scalar.dma_start(
                    frk[:], fr_d[:, k * 2 * T_CORE : (k + 1) * 2 * T_CORE]
                )
                fr_tiles[k] = frk

            c_slabs = {}

            def c_dma(I, eng):
                c_sb = c_pool.tile([D, NJ * D], F16, tag=f"c{I}")
                eng.dma_start(c_sb[:], c2[:, I * NJ * D : (I + 1) * NJ * D])
                c_slabs[I] = c_sb

            fr_dma(0)
            c_dma(0, nc.scalar)
            fr_dma(1)

            ps = psum_pool.tile([D, T_CORE], F32)

            q = 0
            for I in range(NI):
                k = I // 2
                if k not in fr_tiles:
                    fr_dma(k)
                if I % 2 == 0 and (k + 1) not in fr_tiles and k + 1 < NI // 2:
                    fr_dma(k + 1)
                if I + 1 < NI:
                    c_dma(I + 1, nc.sync if I % 2 == 0 else nc.scalar)
                c_sb = c_slabs.pop(I)
                fr_sl = fr_tiles[k][:, (I % 2) * T_CORE : (I % 2 + 1) * T_CORE]

                def f_view(reps):
                    return bass.AP(
                        fr_sl.tensor,
                        fr_sl.offset,
                        [fr_sl.ap[0], [0, reps], [1, T_CORE]],
                    )

                def emit_unit(Jbase, cnt, a_ap, tag=None):
                    nonlocal q
                    if tag is None:
                        pt = p_pool.tile([D, cnt * T_CORE], F16, tag="p")
                    else:
                        pt = const_pool.tile([D, cnt * T_CORE], F16, tag=tag)
                    nc.vector.tensor_mul(pt[:], f_view(cnt), a_ap)
                    for kk in range(cnt):
                        J = Jbase + kk
                        nc.tensor.matmul(
                            ps[:],
                            c_sb[:, J * D : (J + 1) * D],
                            pt[:, kk * T_CORE : (kk + 1) * T_CORE],
                            start=(q == 0),
                            stop=(q == NI * NJ - 1),
                        )
                        q += 1

                if I == 0:
                    # tiny first unit off the small a_j0 tile, then the rest
                    # of the group off the two a halves
                    emit_unit(0, 1, a_j0[:], tag="p0")
                    emit_unit(1, 3, a_half0[:, T_CORE : 4 * T_CORE], tag="p1")
                    emit_unit(4, 4, a_half1[:])
                elif I == NI - 1:
                    # taper the last group so the PE chain ends right after
                    # the final (single-chunk) TT
                    emit_unit(0, 4, a_half0[:])
                    emit_unit(4, 3, a_half1[:, 0 : 3 * T_CORE])
                    emit_unit(7, 1, a_half1[:, 3 * T_CORE : 4 * T_CORE], tag="pz")
                else:
                    emit_unit(0, 4, a_half0[:])
                    emit_unit(4, 4, a_half1[:])

            o_sb = const_pool.tile([D, T_CORE], F32, tag="o")
            nc.vector.tensor_copy(o_sb[:], ps[:])
            nc.sync.dma_start(out_t[:, :], o_sb[:])

    nc.compile()
    return nc


def _get_nc():
    global _NC_CACHE
    if _NC_CACHE is None:
        _NC_CACHE = _build()
    return _NC_CACHE


def _prep_in_maps(func_and_arg, cooccurrences):
    fa = np.asarray(func_and_arg, dtype=np.float32).reshape(T_TOTAL, 2 * D)
    c2r = (
        np.asarray(cooccurrences, dtype=np.float32)
        .reshape(NI, GI, NJ, GJ, D)
        .transpose(1, 3, 0, 2, 4)
        .reshape(D, D * D)
        .astype(NP_F16)
    )
    c2r = np.ascontiguousarray(c2r)
    in_maps = []
    for c in range(N_CORES):
        s = fa[c * T_CORE : (c + 1) * T_CORE]  # [512, 256]
        f_tc = np.ascontiguousarray(s[:, :D].T).astype(NP_F16)  # [128 i, 512 t]
        a_tc = np.ascontiguousarray(s[:, D:].T).astype(NP_F16)  # [128 j, 512 t]
        # fr[p, I*512+t] = f[I*8 + p//16, t]
        fr = np.ascontiguousarray(
            np.broadcast_to(
                f_tc.reshape(NI, GI, 1, T_CORE), (NI, GI, GJ, T_CORE)
            ).transpose(1, 2, 0, 3).reshape(D, NI * T_CORE)
        )
        # ar[p, J*512+t] = a[J*16 + p%16, t]
        ar = np.ascontiguousarray(
            np.broadcast_to(
                a_tc.reshape(1, NJ, GJ, T_CORE).transpose(0, 2, 1, 3),
                (GI, GJ, NJ, T_CORE),
            ).reshape(D, NJ * T_CORE)
        )
        in_maps.append({"fr": fr, "ar": ar, "c2": c2r})
    return in_maps


def kernel(func_and_arg: np.ndarray, cooccurrences: np.ndarray) -> np.ndarray:
    assert func_and_arg.shape == (4, 1024, 2 * D)
    assert cooccurrences.shape == (D, D, D)

    in_maps = _prep_in_maps(func_and_arg, cooccurrences)
    nc = _get_nc()
    res = run_bass_kernel_spmd(nc, in_maps, core_ids=list(range(N_CORES)))

    # out_t per core: [z=128, t=512] -> [t, z]; concat over cores -> [4096, 128]
    outs = [res.results[c]["out_t"].T for c in range(N_CORES)]
    out = np.concatenate(outs, axis=0).reshape(4, 1024, D).astype(np.float32)
    return out
